# revision 67
# baseline (speedup 1.0000x reference)
"""Masked-attention kernel for 8 TRN2 NeuronCores (batch-parallel sharding).

Per-core shard: 2 batches of [S=2048, D=128] Q/K/V + [S, S] bool mask.
Layout strategy (per core):
  - scores are computed TRANSPOSED (S^T[k, q]) so the PV matmul consumes the
    exp() output directly with V in its natural [k, d] layout.
  - the mask is folded into the scores inside the PE accumulation: an extra
    matmul per (k-tile, q-subtile) with the mask chunk (DMA-cast u8->fp8e4)
    as the stationary operand and a -240*I fp8 identity as the moving
    operand; exp() then flushes masked entries to ~0.
  - softmax denominator: DVE accumulates exp tiles across k-tiles, then per
    q-subtile one [acc-chunk]^T @ ones matmul gives the denominator as a
    PSUM column; reciprocal on DVE; applied as a per-partition scalar after
    the final transpose.
  - Q^T/K^T and O^T->O transposes run on the TensorEngine (is_transpose
    matmuls vs a fp16 identity) through a small PSUM staging pool — the DMA
    xbar path was slower here because Tile serializes dma-transposes against
    in-flight DMAs.
  - Q/K loads ride the two HWDGE rings (SP + ACT) in halves, masks ride
    SWDGE (with the u8->fp8 cast) in 512-column chunks, so the first
    matmul can start ~10us in.
"""

import numpy as np
import ml_dtypes

B, S, D = 16, 2048, 128
NCORES = 8
BP = B // NCORES  # batches per core
P = 128
QC = 1024  # q-chunk (columns of the transposed score tile)
NQC = S // QC
NKT = S // P  # k tiles
NQS = QC // P  # q subtiles per chunk
MM_N = 512  # matmul moving free dim
SCALE = 1.0 / float(np.sqrt(128.0))
MASK_NEG = -240.0

_CACHE = {}


def build_nc(loop=True):
    import concourse.mybir as mybir
    import concourse.tile as tile
    from concourse import bacc

    fp16 = mybir.dt.float16
    fp32 = mybir.dt.float32

    nc = bacc.Bacc("TRN2", target_bir_lowering=False, debug=False,
                   num_devices=NCORES)

    Qd = nc.dram_tensor("Q", [BP, S, D], fp32, kind="ExternalInput")
    Kd = nc.dram_tensor("K", [BP, S, D], fp32, kind="ExternalInput")
    Vd = nc.dram_tensor("V", [BP, S, D], fp32, kind="ExternalInput")
    Md = nc.dram_tensor("mask", [BP, S, S], mybir.dt.uint8, kind="ExternalInput")
    if loop:
        # run-count knob for differential HW timing (graded path: loop=False)
        Id = nc.dram_tensor("iters", [1, 1], mybir.dt.int32,
                            kind="ExternalInput")
    Od = nc.dram_tensor("out", [BP, S, D], fp32, kind="ExternalOutput")

    negI_np = (MASK_NEG * np.eye(P, dtype=np.float32)).astype(
        ml_dtypes.float8_e4m3)
    negI_dram = nc.inline_tensor(negI_np, name="negI_const")
    ident_dram = nc.inline_tensor(np.eye(P, dtype=np.float16),
                                  name="ident_const")

    with tile.TileContext(nc) as tc:
        with tc.tile_pool(name="consts", bufs=1) as consts, \
             tc.tile_pool(name="stag", bufs=3) as stag, \
             tc.tile_pool(name="qkv", bufs=1) as qkv, \
             tc.tile_pool(name="maskp", bufs=6) as maskp, \
             tc.tile_pool(name="pp", bufs=3) as pp, \
             tc.tile_pool(name="accp", bufs=2) as accp, \
             tc.tile_pool(name="outp", bufs=2) as outp, \
             tc.tile_pool(name="spsum", bufs=2, space="PSUM") as spsum, \
             tc.tile_pool(name="opsum", bufs=1, space="PSUM") as opsum, \
             tc.tile_pool(name="tpsum", bufs=2, space="PSUM") as tpsum:

            negI = consts.tile([P, P], mybir.dt.float8e4)
            nc.sync.dma_start(out=negI[:, :], in_=negI_dram.ap())
            ident = consts.tile([P, P], fp16)
            nc.sync.dma_start(out=ident[:, :], in_=ident_dram.ap())
            ones_col = consts.tile([P, 1], fp16)
            nc.vector.memset(ones_col, 1.0)

            pools = (stag, qkv, maskp, pp, accp, outp, spsum, opsum, tpsum)
            if loop:
                it_sb = consts.tile([1, 1], mybir.dt.int32)
                nc.sync.dma_start(out=it_sb[:, :], in_=Id.ap())
                n_iters = nc.values_load(it_sb[:, :],
                                         skip_runtime_bounds_check=True)
                with tc.For_i(0, n_iters, 1,
                              hint_engines=(mybir.EngineType.PE,
                                            mybir.EngineType.Activation,
                                            mybir.EngineType.DVE,
                                            mybir.EngineType.SP,
                                            mybir.EngineType.Pool)):
                    _kernel_body(nc, mybir, Qd, Kd, Vd, Md, Od, negI,
                                 ident, ones_col, *pools)
            else:
                _kernel_body(nc, mybir, Qd, Kd, Vd, Md, Od, negI,
                             ident, ones_col, *pools)
    nc.compile()
    return nc


def _kernel_body(nc, mybir, Qd, Kd, Vd, Md, Od, negI, ident, ones_col,
                 stag, qkv, maskp, pp, accp, outp, spsum, opsum, tpsum):
    fp16 = mybir.dt.float16
    fp32 = mybir.dt.float32
    fp8 = mybir.dt.float8e4
    Exp = mybir.ActivationFunctionType.Exp

    MC = 512  # mask column-chunk (k) per DMA

    def load_mask_ck(b, qc, ck):
        # one tile per 512-column chunk: a single writer DMA, so the first
        # consuming matmul doesn't wait on later chunks (tile-granular deps)
        t = maskp.tile([P, NQS, MC], fp8, name="mfck")
        nc.gpsimd.dma_start(
            out=t[:, :, :],
            in_=Md.ap()[b, qc * QC:(qc + 1) * QC, ck * MC:(ck + 1) * MC]
                .rearrange("(s p) k -> p s k", p=P))
        return t

    # ---- prefetch the first mask columns before everything else (SWDGE) ----
    mf00 = [None] * (S // MC)
    mf00[0] = load_mask_ck(0, 0, 0)

    # ---- prep: load (HWDGE) + DVE-cast + PE-transpose Q/K, load V ----
    HT = NKT // 2  # tiles per half-load

    def load_f32_half(src_ap, b, h, ring):
        f = stag.tile([P, HT, D], fp32, name="ldf")
        ring(out=f[:, :, :],
             in_=src_ap[b, h * HT * P:(h + 1) * HT * P, :]
                 .rearrange("(t p) d -> p t d", p=P))
        return f

    def load_cast_half(src_ap, b, h, ring):
        # load a [S/2, D] f32 half and cast it to fp16 staging
        f = load_f32_half(src_ap, b, h, ring)
        g = stag.tile([P, HT, D], fp16, name="ldh")
        nc.vector.tensor_copy(out=g[:, :, :], in_=f[:, :, :])
        return g

    QT4 = HT // 2  # tiles per quarter

    def transpose_quarter(src_ap, dst, b, q4, ring):
        # finer first-quarter pipelining for the very first k-tiles
        f = stag.tile([P, QT4, D], fp32, name="ldf4")
        ring(out=f[:, :, :],
             in_=src_ap[b, q4 * QT4 * P:(q4 + 1) * QT4 * P, :]
                 .rearrange("(t p) d -> p t d", p=P))
        g = stag.tile([P, QT4, D], fp16, name="ldh4")
        nc.vector.tensor_copy(out=g[:, :, :], in_=f[:, :, :])
        tps = tpsum.tile([P, QT4 * P], fp16, name="tps")
        for t in range(QT4):
            nc.tensor.transpose(tps[:, t * P:(t + 1) * P],
                                g[:, t, :], ident[:, :])
        nc.vector.tensor_copy(
            out=dst[:, q4 * QT4 * P:(q4 + 1) * QT4 * P], in_=tps[:, :])

    def transpose_half(g, dst):
        # PE-transpose each 128x128 tile of a staged half into a 1-bank
        # PSUM slot, then copy back to dst [d, s-half]
        tps = tpsum.tile([P, HT * P], fp16, name="tps")
        for t in range(HT):
            nc.tensor.transpose(tps[:, t * P:(t + 1) * P],
                                g[:, t, :], ident[:, :])
        nc.vector.tensor_copy(out=dst[:, :], in_=tps[:, :])

    def prep_batch(b):
        # per-half tiles: a consumer of h0 never waits on h1's writers.
        # h0: load+cast+transpose now; h1: f32 loads now, cast+transpose
        # deferred to mid-k-loop (finish()) so neither the in-order PE nor
        # the DVE FIFO head-of-line blocks the first k-loop.
        ktt = [qkv.tile([P, HT * P], fp16, name=f"ktt{b}{h}")
               for h in range(2)]
        qt = [qkv.tile([P, HT * P], fp16, name=f"qt{b}{h}")
              for h in range(2)]
        vsb = [qkv.tile([P, HT, D], fp16, name=f"vsb{b}{h}")
               for h in range(2)]

        def load_v_half(h):
            vf = stag.tile([P, HT, D], fp32, name="vf")
            nc.sync.dma_start(
                out=vf[:, :, :],
                in_=Vd.ap()[b, h * HT * P:(h + 1) * HT * P, :]
                    .rearrange("(t p) d -> p t d", p=P))
            nc.vector.tensor_copy(out=vsb[h][:, :, :], in_=vf[:, :, :])

        for q4 in range(2):
            transpose_quarter(Kd.ap(), ktt[0], b, q4, nc.sync.dma_start)
            transpose_quarter(Qd.ap(), qt[0], b, q4, nc.scalar.dma_start)
        load_v_half(0)
        if b == 0:
            mf00[1] = load_mask_ck(0, 0, 1)
        fk1 = load_f32_half(Kd.ap(), b, 1, nc.sync.dma_start)
        fq1 = load_f32_half(Qd.ap(), b, 1, nc.scalar.dma_start)
        load_v_half(1)
        if b == 0:
            mf00[2] = load_mask_ck(0, 0, 2)
            mf00[3] = load_mask_ck(0, 0, 3)

        state = {}

        def finish_cast():
            gk1 = stag.tile([P, HT, D], fp16, name="ldh")
            nc.vector.tensor_copy(out=gk1[:, :, :], in_=fk1[:, :, :])
            gq1 = stag.tile([P, HT, D], fp16, name="ldh")
            nc.vector.tensor_copy(out=gq1[:, :, :], in_=fq1[:, :, :])
            state["g"] = (gk1, gq1)

        def finish_transpose():
            gk1, gq1 = state["g"]
            transpose_half(gk1, ktt[1])
            transpose_half(gq1, qt[1])
        return qt, ktt, vsb, (finish_cast, finish_transpose)

    prepped = {0: prep_batch(0)}
    finished = set()

    # ---- main flash loop over (batch, q-chunk, k-tile) ----
    for b in range(BP):
        for qc in range(NQC):
            if (b, qc) == (0, 1) and BP > 1:
                prepped[1] = prep_batch(1)
            qt, ktt, vsb, finish_fns = prepped[b]
            if b == 0 and qc == 0:
                mf = mf00
            else:
                mf = [load_mask_ck(b, qc, ck) for ck in range(S // MC)]
            acc = accp.tile([P, QC], fp16, name="acc")
            ops = opsum.tile([P, QC], fp32, name="opsum")
            for kt in range(NKT):
                if kt == HT - 3 and b not in finished:
                    finish_fns[0]()
                if kt == HT - 1 and b not in finished:
                    finish_fns[1]()
                    finished.add(b)
                sc = spsum.tile([P, QC], fp32, name="scores")
                mfck = mf[kt * P // MC]
                kcol = (kt * P) % MC
                for sq in range(NQS):
                    # start=True only on the first matmul touching each PSUM
                    # bank (start clears the whole bank's has_written bits)
                    nc.tensor.matmul(
                        sc[:, sq * P:(sq + 1) * P],
                        lhsT=mfck[:, sq, kcol:kcol + P],
                        rhs=negI[:, :],
                        start=(sq % (MM_N // P) == 0), stop=False,
                        skip_group_check=True)
                kh, kloc = kt // HT, (kt % HT) * P
                for n in range(0, QC, MM_N):
                    nc.tensor.matmul(
                        sc[:, n:n + MM_N],
                        lhsT=ktt[kh][:, kloc:kloc + P],
                        rhs=qt[qc][:, n:n + MM_N],
                        start=False, stop=True, skip_group_check=True)
                pt = pp.tile([P, QC], fp16, name="pt")
                nc.scalar.activation(out=pt[:, :], in_=sc[:, :],
                                     func=Exp, scale=SCALE)
                if kt == 0:
                    nc.vector.tensor_copy(out=acc[:, :], in_=pt[:, :])
                else:
                    nc.vector.tensor_add(out=acc[:, :], in0=acc[:, :],
                                         in1=pt[:, :])
                # PV lags one k-tile so the PE never waits on exp(kt)
                if kt > 0:
                    j = kt - 1
                    for n in range(0, QC, MM_N):
                        nc.tensor.matmul(
                            ops[:, n:n + MM_N],
                            lhsT=vsb[j // HT][:, j % HT, :],
                            rhs=prev_pt[:, n:n + MM_N],
                            start=(kt == 1), stop=False,
                            skip_group_check=True)
                prev_pt = pt
            j = NKT - 1
            for n in range(0, QC, MM_N):
                nc.tensor.matmul(
                    ops[:, n:n + MM_N],
                    lhsT=vsb[j // HT][:, j % HT, :],
                    rhs=prev_pt[:, n:n + MM_N],
                    start=False, stop=True,
                    skip_group_check=True)

            # denominator as a PSUM column per q-subtile:
            # den[q_local, sq] = sum_k acc[k, sq*128 + q_local]
            den = tpsum.tile([P, NQS], fp32, name="tps")
            for sq in range(NQS):
                nc.tensor.matmul(den[:, sq:sq + 1],
                                 lhsT=acc[:, sq * P:(sq + 1) * P],
                                 rhs=ones_col[:, :],
                                 start=True, stop=True,
                                 skip_group_check=True)
            rcol = outp.tile([P, NQS], fp32, name="rcol")
            nc.vector.reciprocal(out=rcol[:, :], in_=den[:, :])

            # epilogue in two 512-col halves so copy/transpose/scale/store
            # pipeline (shorter serial tail on the final chunk)
            HQ = NQS // 2
            for hh in range(2):
                ot = outp.tile([P, HQ * P], fp16, name="ot")
                nc.scalar.copy(out=ot[:, :],
                               in_=ops[:, hh * HQ * P:(hh + 1) * HQ * P])
                osb = tpsum.tile([P, HQ * P], fp16, name="tps")
                for t in range(HQ):
                    nc.tensor.transpose(osb[:, t * P:(t + 1) * P],
                                        ot[:, t * P:(t + 1) * P],
                                        ident[:, :])
                osf = outp.tile([P, HQ, D], fp32, name="osf")
                for t in range(HQ):
                    nc.vector.tensor_scalar_mul(
                        out=osf[:, t, :],
                        in0=osb[:, t * P:(t + 1) * P],
                        scalar1=rcol[:, hh * HQ + t:hh * HQ + t + 1])
                ring = nc.scalar.dma_start if hh == 0 else nc.sync.dma_start
                ring(out=Od.ap()[b,
                                 qc * QC + hh * HQ * P:
                                 qc * QC + (hh + 1) * HQ * P, :]
                     .rearrange("(t p) d -> p t d", p=P),
                     in_=osf[:, :, :])


def _get_nc(loop=False):
    key = f"nc_loop{loop}"
    if key not in _CACHE:
        _CACHE[key] = build_nc(loop=loop)
    return _CACHE[key]


def kernel(Q, K, V, mask, dk=128):
    from concourse.bass_utils import run_bass_kernel_spmd

    assert int(dk) == 128
    Q = np.ascontiguousarray(np.asarray(Q, dtype=np.float32))
    K = np.ascontiguousarray(np.asarray(K, dtype=np.float32))
    V = np.ascontiguousarray(np.asarray(V, dtype=np.float32))
    mask_u8 = np.ascontiguousarray(np.asarray(mask)).astype(np.uint8)

    nc = _get_nc(loop=False)
    in_maps = []
    for c in range(NCORES):
        sl = slice(c * BP, (c + 1) * BP)
        in_maps.append({
            "Q": np.ascontiguousarray(Q[sl]),
            "K": np.ascontiguousarray(K[sl]),
            "V": np.ascontiguousarray(V[sl]),
            "mask": np.ascontiguousarray(mask_u8[sl]),
        })
    res = run_bass_kernel_spmd(nc, in_maps, core_ids=list(range(NCORES)))
    return np.concatenate([r["out"] for r in res.results], axis=0)


# revision 68
# speedup vs baseline: 1.0715x; 1.0715x over previous
"""Masked-attention kernel for 8 TRN2 NeuronCores (batch-parallel sharding).

Per-core shard: 2 batches of [S=2048, D=128] Q/K/V + [S, S] bool mask.
Layout strategy (per core):
  - scores are computed TRANSPOSED (S^T[k, q]) so the PV matmul consumes the
    exp() output directly with V in its natural [k, d] layout.
  - the mask is folded into the scores inside the PE accumulation: an extra
    matmul per (k-tile, q-subtile) with the mask chunk (DMA-cast u8->fp8e4)
    as the stationary operand and a -240*I fp8 identity as the moving
    operand; exp() then flushes masked entries to ~0.
  - softmax denominator: DVE accumulates exp tiles across k-tiles, then per
    q-subtile one [acc-chunk]^T @ ones matmul gives the denominator as a
    PSUM column; reciprocal on DVE; applied as a per-partition scalar after
    the final transpose.
  - Q^T/K^T and O^T->O transposes run on the TensorEngine (is_transpose
    matmuls vs a fp16 identity) through a small PSUM staging pool — the DMA
    xbar path was slower here because Tile serializes dma-transposes against
    in-flight DMAs.
  - Q/K loads ride the two HWDGE rings (SP + ACT) in halves, masks ride
    SWDGE (with the u8->fp8 cast) in 512-column chunks, so the first
    matmul can start ~10us in.
"""

import numpy as np
import ml_dtypes

B, S, D = 16, 2048, 128
NCORES = 8
BP = B // NCORES  # batches per core
P = 128
QC = 1024  # q-chunk (columns of the transposed score tile)
NQC = S // QC
NKT = S // P  # k tiles
NQS = QC // P  # q subtiles per chunk
MM_N = 512  # matmul moving free dim
SCALE = 1.0 / float(np.sqrt(128.0))
MASK_NEG = -240.0

_CACHE = {}


def build_nc(loop=True):
    import concourse.mybir as mybir
    import concourse.tile as tile
    from concourse import bacc

    fp16 = mybir.dt.float16
    fp32 = mybir.dt.float32

    nc = bacc.Bacc("TRN2", target_bir_lowering=False, debug=False,
                   num_devices=NCORES)

    Qd = nc.dram_tensor("Q", [BP, S, D], fp32, kind="ExternalInput")
    Kd = nc.dram_tensor("K", [BP, S, D], fp32, kind="ExternalInput")
    Vd = nc.dram_tensor("V", [BP, S, D], fp32, kind="ExternalInput")
    Md = nc.dram_tensor("mask", [BP, S, S], mybir.dt.uint8, kind="ExternalInput")
    if loop:
        # run-count knob for differential HW timing (graded path: loop=False)
        Id = nc.dram_tensor("iters", [1, 1], mybir.dt.int32,
                            kind="ExternalInput")
    Od = nc.dram_tensor("out", [BP, S, D], fp32, kind="ExternalOutput")

    negI_np = (MASK_NEG * np.eye(P, dtype=np.float32)).astype(
        ml_dtypes.float8_e4m3)
    negI_dram = nc.inline_tensor(negI_np, name="negI_const")
    ident_dram = nc.inline_tensor(np.eye(P, dtype=np.float16),
                                  name="ident_const")

    with tile.TileContext(nc) as tc:
        with tc.tile_pool(name="consts", bufs=1) as consts, \
             tc.tile_pool(name="stag", bufs=3) as stag, \
             tc.tile_pool(name="qkv", bufs=1) as qkv, \
             tc.tile_pool(name="maskp", bufs=6) as maskp, \
             tc.tile_pool(name="pp", bufs=3) as pp, \
             tc.tile_pool(name="accp", bufs=2) as accp, \
             tc.tile_pool(name="outp", bufs=2) as outp, \
             tc.tile_pool(name="spsum", bufs=2, space="PSUM") as spsum, \
             tc.tile_pool(name="opsum", bufs=1, space="PSUM") as opsum, \
             tc.tile_pool(name="tpsum", bufs=2, space="PSUM") as tpsum:

            negI = consts.tile([P, P], mybir.dt.float8e4)
            nc.sync.dma_start(out=negI[:, :], in_=negI_dram.ap())
            ident = consts.tile([P, P], fp16)
            nc.sync.dma_start(out=ident[:, :], in_=ident_dram.ap())
            ones_col = consts.tile([P, 1], fp16)
            nc.vector.memset(ones_col, 1.0)

            pools = (stag, qkv, maskp, pp, accp, outp, spsum, opsum, tpsum)
            if loop:
                it_sb = consts.tile([1, 1], mybir.dt.int32)
                nc.sync.dma_start(out=it_sb[:, :], in_=Id.ap())
                n_iters = nc.values_load(it_sb[:, :],
                                         skip_runtime_bounds_check=True)
                with tc.For_i(0, n_iters, 1,
                              hint_engines=(mybir.EngineType.PE,
                                            mybir.EngineType.Activation,
                                            mybir.EngineType.DVE,
                                            mybir.EngineType.SP,
                                            mybir.EngineType.Pool)):
                    _kernel_body(nc, mybir, Qd, Kd, Vd, Md, Od, negI,
                                 ident, ones_col, *pools)
            else:
                _kernel_body(nc, mybir, Qd, Kd, Vd, Md, Od, negI,
                             ident, ones_col, *pools)
    nc.compile()
    return nc


def _kernel_body(nc, mybir, Qd, Kd, Vd, Md, Od, negI, ident, ones_col,
                 stag, qkv, maskp, pp, accp, outp, spsum, opsum, tpsum):
    fp16 = mybir.dt.float16
    fp32 = mybir.dt.float32
    fp8 = mybir.dt.float8e4
    Exp = mybir.ActivationFunctionType.Exp

    MC = 512  # mask column-chunk (k) per DMA

    def load_mask_ck(b, qc, ck):
        # one tile per 512-column chunk: a single writer DMA, so the first
        # consuming matmul doesn't wait on later chunks (tile-granular deps)
        t = maskp.tile([P, NQS, MC], fp8, name="mfck")
        nc.gpsimd.dma_start(
            out=t[:, :, :],
            in_=Md.ap()[b, qc * QC:(qc + 1) * QC, ck * MC:(ck + 1) * MC]
                .rearrange("(s p) k -> p s k", p=P))
        return t

    # ---- prefetch the first mask columns before everything else (SWDGE) ----
    mf00 = [None] * (S // MC)
    mf00[0] = load_mask_ck(0, 0, 0)

    # ---- prep: load (HWDGE) + DVE-cast + PE-transpose Q/K, load V ----
    HT = NKT // 2  # tiles per half-load

    def load_f32_half(src_ap, b, h, ring):
        f = stag.tile([P, HT, D], fp32, name="ldf")
        ring(out=f[:, :, :],
             in_=src_ap[b, h * HT * P:(h + 1) * HT * P, :]
                 .rearrange("(t p) d -> p t d", p=P))
        return f

    def load_cast_half(src_ap, b, h, ring):
        # load a [S/2, D] f32 half and cast it to fp16 staging
        f = load_f32_half(src_ap, b, h, ring)
        g = stag.tile([P, HT, D], fp16, name="ldh")
        nc.vector.tensor_copy(out=g[:, :, :], in_=f[:, :, :])
        return g

    QT4 = HT // 2  # tiles per quarter

    def transpose_quarter(src_ap, dst, b, q4, ring):
        # finer first-quarter pipelining for the very first k-tiles;
        # dst is a single-writer per-quarter tile [P, 512]
        f = stag.tile([P, QT4, D], fp32, name="ldf4")
        ring(out=f[:, :, :],
             in_=src_ap[b, q4 * QT4 * P:(q4 + 1) * QT4 * P, :]
                 .rearrange("(t p) d -> p t d", p=P))
        g = stag.tile([P, QT4, D], fp16, name="ldh4")
        nc.vector.tensor_copy(out=g[:, :, :], in_=f[:, :, :])
        tps = tpsum.tile([P, QT4 * P], fp16, name="tps")
        for t in range(QT4):
            nc.tensor.transpose(tps[:, t * P:(t + 1) * P],
                                g[:, t, :], ident[:, :])
        nc.vector.tensor_copy(out=dst[:, :], in_=tps[:, :])

    def transpose_half(g, dstA, dstB):
        # PE-transpose a staged half into two per-quarter tiles
        tps = tpsum.tile([P, HT * P], fp16, name="tps")
        for t in range(HT):
            nc.tensor.transpose(tps[:, t * P:(t + 1) * P],
                                g[:, t, :], ident[:, :])
        nc.vector.tensor_copy(out=dstA[:, :], in_=tps[:, :QT4 * P])
        nc.vector.tensor_copy(out=dstB[:, :], in_=tps[:, QT4 * P:])

    def prep_batch(b):
        # per-half tiles: a consumer of h0 never waits on h1's writers.
        # h0: load+cast+transpose now; h1: f32 loads now, cast+transpose
        # deferred to mid-k-loop (finish()) so neither the in-order PE nor
        # the DVE FIFO head-of-line blocks the first k-loop.
        ktt = [qkv.tile([P, QT4 * P], fp16, name=f"ktt{b}{q4}")
               for q4 in range(4)]
        qt = [qkv.tile([P, QT4 * P], fp16, name=f"qt{b}{q4}")
              for q4 in range(4)]
        vsb = [qkv.tile([P, HT, D], fp16, name=f"vsb{b}{h}")
               for h in range(2)]

        def load_v_half(h):
            vf = stag.tile([P, HT, D], fp32, name="vf")
            nc.sync.dma_start(
                out=vf[:, :, :],
                in_=Vd.ap()[b, h * HT * P:(h + 1) * HT * P, :]
                    .rearrange("(t p) d -> p t d", p=P))
            nc.vector.tensor_copy(out=vsb[h][:, :, :], in_=vf[:, :, :])

        for q4 in range(2):
            transpose_quarter(Kd.ap(), ktt[q4], b, q4, nc.sync.dma_start)
            transpose_quarter(Qd.ap(), qt[q4], b, q4, nc.scalar.dma_start)
        load_v_half(0)
        if b == 0:
            mf00[1] = load_mask_ck(0, 0, 1)
        fk1 = load_f32_half(Kd.ap(), b, 1, nc.sync.dma_start)
        fq1 = load_f32_half(Qd.ap(), b, 1, nc.scalar.dma_start)
        load_v_half(1)
        if b == 0:
            mf00[2] = load_mask_ck(0, 0, 2)
            mf00[3] = load_mask_ck(0, 0, 3)

        state = {}

        def finish_cast():
            gk1 = stag.tile([P, HT, D], fp16, name="ldh")
            nc.vector.tensor_copy(out=gk1[:, :, :], in_=fk1[:, :, :])
            gq1 = stag.tile([P, HT, D], fp16, name="ldh")
            nc.vector.tensor_copy(out=gq1[:, :, :], in_=fq1[:, :, :])
            state["g"] = (gk1, gq1)

        def finish_transpose():
            gk1, gq1 = state["g"]
            transpose_half(gk1, ktt[2], ktt[3])
            transpose_half(gq1, qt[2], qt[3])
        return qt, ktt, vsb, (finish_cast, finish_transpose)

    prepped = {0: prep_batch(0)}
    finished = set()

    # ---- main flash loop over (batch, q-chunk, k-tile) ----
    for b in range(BP):
        for qc in range(NQC):
            if (b, qc) == (0, 1) and BP > 1:
                prepped[1] = prep_batch(1)
            qt, ktt, vsb, finish_fns = prepped[b]
            if b == 0 and qc == 0:
                mf = mf00
            else:
                mf = [load_mask_ck(b, qc, ck) for ck in range(S // MC)]
            acc = accp.tile([P, QC], fp16, name="acc")
            ops = opsum.tile([P, QC], fp32, name="opsum")
            for kt in range(NKT):
                if kt == HT - 3 and b not in finished:
                    finish_fns[0]()
                if kt == HT - 1 and b not in finished:
                    finish_fns[1]()
                    finished.add(b)
                sc = spsum.tile([P, QC], fp32, name="scores")
                mfck = mf[kt * P // MC]
                kcol = (kt * P) % MC
                for sq in range(NQS):
                    # start=True only on the first matmul touching each PSUM
                    # bank (start clears the whole bank's has_written bits)
                    nc.tensor.matmul(
                        sc[:, sq * P:(sq + 1) * P],
                        lhsT=mfck[:, sq, kcol:kcol + P],
                        rhs=negI[:, :],
                        start=(sq % (MM_N // P) == 0), stop=False,
                        skip_group_check=True)
                kh, kloc = kt // QT4, (kt % QT4) * P
                for n in range(0, QC, MM_N):
                    nc.tensor.matmul(
                        sc[:, n:n + MM_N],
                        lhsT=ktt[kh][:, kloc:kloc + P],
                        rhs=qt[qc * 2 + n // MM_N][:, :],
                        start=False, stop=True, skip_group_check=True)
                pt = pp.tile([P, QC], fp16, name="pt")
                nc.scalar.activation(out=pt[:, :], in_=sc[:, :],
                                     func=Exp, scale=SCALE)
                if kt == 0:
                    nc.vector.tensor_copy(out=acc[:, :], in_=pt[:, :])
                else:
                    nc.vector.tensor_add(out=acc[:, :], in0=acc[:, :],
                                         in1=pt[:, :])
                # PV lags one k-tile so the PE never waits on exp(kt)
                if kt > 0:
                    j = kt - 1
                    for n in range(0, QC, MM_N):
                        nc.tensor.matmul(
                            ops[:, n:n + MM_N],
                            lhsT=vsb[j // HT][:, j % HT, :],
                            rhs=prev_pt[:, n:n + MM_N],
                            start=(kt == 1), stop=False,
                            skip_group_check=True)
                prev_pt = pt
            j = NKT - 1
            for n in range(0, QC, MM_N):
                nc.tensor.matmul(
                    ops[:, n:n + MM_N],
                    lhsT=vsb[j // HT][:, j % HT, :],
                    rhs=prev_pt[:, n:n + MM_N],
                    start=False, stop=True,
                    skip_group_check=True)

            # denominator as a PSUM column per q-subtile:
            # den[q_local, sq] = sum_k acc[k, sq*128 + q_local]
            den = tpsum.tile([P, NQS], fp32, name="tps")
            for sq in range(NQS):
                nc.tensor.matmul(den[:, sq:sq + 1],
                                 lhsT=acc[:, sq * P:(sq + 1) * P],
                                 rhs=ones_col[:, :],
                                 start=True, stop=True,
                                 skip_group_check=True)
            rcol = outp.tile([P, NQS], fp32, name="rcol")
            nc.vector.reciprocal(out=rcol[:, :], in_=den[:, :])

            # epilogue in two 512-col halves so copy/transpose/scale/store
            # pipeline (shorter serial tail on the final chunk)
            HQ = NQS // 2
            for hh in range(2):
                ot = outp.tile([P, HQ * P], fp16, name="ot")
                nc.scalar.copy(out=ot[:, :],
                               in_=ops[:, hh * HQ * P:(hh + 1) * HQ * P])
                osb = tpsum.tile([P, HQ * P], fp16, name="tps")
                for t in range(HQ):
                    nc.tensor.transpose(osb[:, t * P:(t + 1) * P],
                                        ot[:, t * P:(t + 1) * P],
                                        ident[:, :])
                osf = outp.tile([P, HQ, D], fp32, name="osf")
                for t in range(HQ):
                    nc.vector.tensor_scalar_mul(
                        out=osf[:, t, :],
                        in0=osb[:, t * P:(t + 1) * P],
                        scalar1=rcol[:, hh * HQ + t:hh * HQ + t + 1])
                ring = nc.scalar.dma_start if hh == 0 else nc.sync.dma_start
                ring(out=Od.ap()[b,
                                 qc * QC + hh * HQ * P:
                                 qc * QC + (hh + 1) * HQ * P, :]
                     .rearrange("(t p) d -> p t d", p=P),
                     in_=osf[:, :, :])


def _get_nc(loop=False):
    key = f"nc_loop{loop}"
    if key not in _CACHE:
        _CACHE[key] = build_nc(loop=loop)
    return _CACHE[key]


def kernel(Q, K, V, mask, dk=128):
    from concourse.bass_utils import run_bass_kernel_spmd

    assert int(dk) == 128
    Q = np.ascontiguousarray(np.asarray(Q, dtype=np.float32))
    K = np.ascontiguousarray(np.asarray(K, dtype=np.float32))
    V = np.ascontiguousarray(np.asarray(V, dtype=np.float32))
    mask_u8 = np.ascontiguousarray(np.asarray(mask)).astype(np.uint8)

    nc = _get_nc(loop=False)
    in_maps = []
    for c in range(NCORES):
        sl = slice(c * BP, (c + 1) * BP)
        in_maps.append({
            "Q": np.ascontiguousarray(Q[sl]),
            "K": np.ascontiguousarray(K[sl]),
            "V": np.ascontiguousarray(V[sl]),
            "mask": np.ascontiguousarray(mask_u8[sl]),
        })
    res = run_bass_kernel_spmd(nc, in_maps, core_ids=list(range(NCORES)))
    return np.concatenate([r["out"] for r in res.results], axis=0)
